# revision 12
# baseline (speedup 1.0000x reference)
"""DNRI MLP decoder (GNN message passing) Trainium2 Bass kernel.

Sharding: data-parallel over batch (B=512 -> 64 per core x 8 cores).

Per-core device pipeline (all value arithmetic on device):
  - Node projections A = X @ m1_w[:D], B = X @ m1_w[D:] via one matmul per
    elem-pair using a host-prepared block-sparse X^T layout (xt4).
  - Edge gather Hpre'[b, (i,j)] = A[b, j] + B[b, i] over the dense V x V grid
    via matmul against a static one-hot matrix G.  Real edges map to grid
    cells (send, recv); edge weights land in w_grid (0 elsewhere), so absent
    cells and skipped (diagonal) cells contribute nothing downstream.
  - h = relu(Hpre' + b1)  (ScalarE drain)
  - msg = relu(h @ m2_w + b2)  (block-diag m2, ScalarE drain)
  - msgw = msg * w_grid-broadcast  (VectorE; edge weight >= 0 so exact)
  - agg[b, j] = sum_i msgw[b, (i,j)] : 8 PSUM-accumulating identity matmuls
    fold i-blocks mod 8, then a strided VectorE reduce finishes the sum.
  - aug = [X, agg]; fc1/fc2/mu/ls heads + tanh/softplus/logp epilogue.

Host does layout marshalling only (transposes, one-hot patterns, padding,
sharding); every floating-point op on tensor values runs on the NeuronCores.
(The only exception is summing duplicate-edge weights during w_grid
staging, which never triggers for the complete-graph edge list.)
"""

import os
import sys

import numpy as np

for _p in ("/opt/trn_rl_repo", "/root/.axon_site/_ro/trn_rl_repo"):
    if os.path.isdir(_p) and _p not in sys.path:
        sys.path.insert(0, _p)

import concourse.bass as bass
import concourse.mybir as mybir
from concourse import bacc, tile
from concourse.bass_utils import run_bass_kernel_spmd

B, V, D, H, A = 512, 64, 32, 64, 8
NCORES = 8
BC = B // NCORES            # 64 batch elems per core
NPAIR = BC // 2             # 32 elem pairs per core
GRID = V * V                # 4096 dense edge grid
TOK = BC * V                # 4096 tokens per core
NGRP = TOK // 512           # 8 token groups of 512
HALF_LOG_2PI = 0.5 * float(np.log(2.0 * np.pi))
LOG2 = float(np.log(2.0))

F32 = mybir.dt.float32
F32R = mybir.dt.float32r
AF = mybir.ActivationFunctionType
ALU = mybir.AluOpType

# matmul input dtype knob (float32r = full-rate fp32 path on TRN2 PE)
MM_DT = F32R


def _mm(npc, out, lhsT, rhs, **kw):
    npc.tensor.matmul(out, lhsT.bitcast(MM_DT), rhs.bitcast(MM_DT), **kw)


def build_nc():
    """Build the per-core Bass module (identical on all cores)."""
    nc = bacc.Bacc()

    # --- DRAM parameters -------------------------------------------------
    xt4_d = nc.declare_dram_parameter("xt4", [128, 128 * NPAIR], F32, isOutput=False)
    xtc_d = nc.declare_dram_parameter("xtc", [D, TOK], F32, isOutput=False)
    wg_d = nc.declare_dram_parameter("wg", [BC, GRID], F32, isOutput=False)
    g_d = nc.declare_dram_parameter("gmat", [128, GRID], F32, isOutput=False)
    wrws_d = nc.declare_dram_parameter("wrws", [128, 128], F32, isOutput=False)
    m2bd_d = nc.declare_dram_parameter("m2bd", [128, 128], F32, isOutput=False)
    i128_d = nc.declare_dram_parameter("i128", [128, 128], F32, isOutput=False)
    fc1w_d = nc.declare_dram_parameter("fc1w", [D + H, H], F32, isOutput=False)
    fc2w_d = nc.declare_dram_parameter("fc2w", [H, H], F32, isOutput=False)
    hdw_d = nc.declare_dram_parameter("hdw", [H, 2 * A], F32, isOutput=False)
    lsum_d = nc.declare_dram_parameter("lsum", [128, 2 * NGRP], F32, isOutput=False)
    bias_d = nc.declare_dram_parameter("biasv", [128, 8], F32, isOutput=False)

    augt_o = nc.declare_dram_parameter("augt", [D + H, TOK], F32, isOutput=True)
    th_o = nc.declare_dram_parameter("tanhpk", [128, 512], F32, isOutput=True)
    lp_o = nc.declare_dram_parameter("logppk", [NGRP, 512], F32, isOutput=True)

    with tile.TileContext(nc) as tc, \
         tc.tile_pool(name="const", bufs=1) as cpool, \
         tc.tile_pool(name="big", bufs=1) as bpool, \
         tc.tile_pool(name="work", bufs=2) as wpool, \
         tc.tile_pool(name="ph2", bufs=2) as ppool, \
         tc.tile_pool(name="hps", bufs=1, space="PSUM") as hps, \
         tc.tile_pool(name="m2ps", bufs=1, space="PSUM") as m2ps, \
         tc.tile_pool(name="scps", bufs=1, space="PSUM") as scps, \
         tc.tile_pool(name="p2ps", bufs=1, space="PSUM") as p2ps:

        # --- constants into SBUF ----------------------------------------
        g_sb = cpool.tile([128, GRID], F32, name="g_sb")
        nc.sync.dma_start(out=g_sb[:], in_=g_d[:])
        xt4_sb = cpool.tile([128, 128 * NPAIR], F32, name="xt4_sb")
        nc.sync.dma_start(out=xt4_sb[:], in_=xt4_d[:])
        wg_sb = cpool.tile([BC, GRID], F32, name="wg_sb")
        nc.sync.dma_start(out=wg_sb[:], in_=wg_d[:])
        wrws_sb = cpool.tile([128, 128], F32, name="wrws_sb")
        nc.sync.dma_start(out=wrws_sb[:], in_=wrws_d[:])
        m2bd_sb = cpool.tile([128, 128], F32, name="m2bd_sb")
        nc.sync.dma_start(out=m2bd_sb[:], in_=m2bd_d[:])
        i128_sb = cpool.tile([128, 128], F32, name="i128_sb")
        nc.sync.dma_start(out=i128_sb[:], in_=i128_d[:])
        fc1w_sb = cpool.tile([D + H, H], F32, name="fc1w_sb")
        nc.sync.dma_start(out=fc1w_sb[:], in_=fc1w_d[:])
        fc2w_sb = cpool.tile([H, H], F32, name="fc2w_sb")
        nc.sync.dma_start(out=fc2w_sb[:], in_=fc2w_d[:])
        hdw_sb = cpool.tile([H, 2 * A], F32, name="hdw_sb")
        nc.sync.dma_start(out=hdw_sb[:], in_=hdw_d[:])
        lsum_sb = cpool.tile([128, 2 * NGRP], F32, name="lsum_sb")
        nc.sync.dma_start(out=lsum_sb[:], in_=lsum_d[:])
        bias_sb = cpool.tile([128, 8], F32, name="bias_sb")
        nc.sync.dma_start(out=bias_sb[:], in_=bias_d[:])

        ab_sb = bpool.tile([128, 128 * NPAIR], F32, name="ab_sb")
        augt_sb = bpool.tile([D + H, TOK], F32, name="augt_sb")
        hp_sb = bpool.tile([128, 512], F32, name="hp_sb")
        # aug rows 0:32 = X features (token order matches column order)
        nc.sync.dma_start(out=augt_sb[0:D, :], in_=xtc_d[:])

        # --- node projections: AB_all[:, 128p:128p+128] per pair --------
        for gidx in range(NGRP):
            pps = m2ps.tile([128, 512], F32, tag="m2", name=f"pj{gidx}")
            for q in range(4):
                p = 4 * gidx + q
                _mm(nc, pps[:, 128 * q:128 * (q + 1)],
                    xt4_sb[:, 128 * p:128 * (p + 1)], wrws_sb[:],
                    start=True, stop=True)
            nc.scalar.activation(ab_sb[:, 512 * gidx:512 * (gidx + 1)], pps[:],
                                 AF.Copy)

        # --- main edge pipeline, one elem-pair at a time ----------------
        for p in range(NPAIR):
            ab_pair = ab_sb[:, 128 * p:128 * (p + 1)]

            # broadcast the two edge-weight rows across partition halves by
            # log-doubling SBUF->SBUF DMAs
            w2_sb = wpool.tile([128, GRID], F32, tag="w2", bufs=1,
                               name=f"w2_{p}")
            nc.sync.dma_start(out=w2_sb[0:1, :], in_=wg_sb[2 * p:2 * p + 1, :])
            nc.sync.dma_start(out=w2_sb[64:65, :],
                              in_=wg_sb[2 * p + 1:2 * p + 2, :])
            for k in (1, 2, 4, 8, 16, 32):
                nc.sync.dma_start(out=w2_sb[k:2 * k, :], in_=w2_sb[0:k, :])
                nc.sync.dma_start(out=w2_sb[64 + k:64 + 2 * k, :],
                                  in_=w2_sb[64:64 + k, :])

            h_sb = wpool.tile([128, GRID], F32, tag="h", name=f"h_{p}")
            for half in range(2):
                hpt = hps.tile([128, 2048], F32, tag="hps", name=f"hp{p}_{half}")
                for cc in range(4):
                    c = 4 * half + cc
                    _mm(nc, hpt[:, 512 * cc:512 * (cc + 1)], ab_pair,
                        g_sb[:, 512 * c:512 * (c + 1)], start=True, stop=True)
                nc.scalar.activation(h_sb[:, 2048 * half:2048 * (half + 1)],
                                     hpt[:], AF.Relu, bias=bias_sb[:, 0:1])

            msg_sb = wpool.tile([128, GRID], F32, tag="msg", name=f"msg_{p}")
            for dh in range(4):
                mpt = m2ps.tile([128, 1024], F32, tag="m2", name=f"mp{p}_{dh}")
                for cc in range(2):
                    c = 2 * dh + cc
                    _mm(nc, mpt[:, 512 * cc:512 * (cc + 1)], m2bd_sb[:],
                        h_sb[:, 512 * c:512 * (c + 1)], start=True, stop=True)
                nc.scalar.activation(msg_sb[:, 1024 * dh:1024 * (dh + 1)],
                                     mpt[:], AF.Relu, bias=bias_sb[:, 1:2])

            # msgw = msg * w (in place)
            nc.vector.tensor_tensor(out=msg_sb[:], in0=msg_sb[:],
                                    in1=w2_sb[:], op=ALU.mult)

            # scatter-sum over i: 8 accumulating identity matmuls fold the
            # 64 i-blocks down to 8, then reduce the remaining 8.
            scp = scps.tile([128, 512], F32, tag="sc", name=f"sc_{p}")
            for c in range(8):
                _mm(nc, scp[:], i128_sb[:], msg_sb[:, 512 * c:512 * (c + 1)],
                    start=(c == 0), stop=(c == 7))
            aggsb = wpool.tile([128, V], F32, tag="agg", name=f"agg_{p}")
            nc.vector.tensor_reduce(
                out=aggsb[:],
                in_=scp[:].rearrange("p (a j) -> p j a", a=8),
                axis=mybir.AxisListType.X, op=ALU.add)

            # relocate agg halves into aug rows 32:96 (partition shift by DMA)
            nc.sync.dma_start(out=augt_sb[D:D + H, 128 * p:128 * p + 64],
                              in_=aggsb[0:64, :])
            nc.sync.dma_start(out=augt_sb[D:D + H, 128 * p + 64:128 * p + 128],
                              in_=aggsb[64:128, :])

            # --- phase 2 on each finished 512-token group ---------------
            if p % 4 == 3:
                g = p // 4
                ps1 = p2ps.tile([64, 512], F32, tag="p2", name=f"f1{g}")
                _mm(nc, ps1[0:64, :], fc1w_sb[:],
                    augt_sb[:, 512 * g:512 * (g + 1)], start=True, stop=True)
                h1 = ppool.tile([64, 512], F32, tag="h1", name=f"h1_{g}")
                nc.scalar.activation(h1[:], ps1[0:64, :], AF.Relu,
                                     bias=bias_sb[0:64, 2:3])
                ps2 = p2ps.tile([64, 512], F32, tag="p2", name=f"f2{g}")
                _mm(nc, ps2[0:64, :], fc2w_sb[:], h1[:], start=True, stop=True)
                h2 = ppool.tile([64, 512], F32, tag="h2", name=f"h2_{g}")
                nc.scalar.activation(h2[:], ps2[0:64, :], AF.Relu,
                                     bias=bias_sb[0:64, 3:4])
                ps3 = p2ps.tile([64, 512], F32, tag="p2", name=f"hd{g}")
                _mm(nc, ps3[0:2 * A, :], hdw_sb[:], h2[:], start=True, stop=True)
                hd = ppool.tile([2 * A, 512], F32, tag="hd", name=f"hd_{g}")
                nc.scalar.activation(hd[:], ps3[0:2 * A, :], AF.Identity,
                                     bias=bias_sb[0:2 * A, 4:5])
                # repack into [128, 512]: group g -> partitions 16g:16g+16
                nc.sync.dma_start(out=hp_sb[16 * g:16 * (g + 1), :], in_=hd[:])

        # --- epilogue: tanh / softplus / clip / logp --------------------
        th = ppool.tile([128, 512], F32, tag="th", bufs=1, name="th")
        nc.scalar.activation(th[:], hp_sb[:], AF.Tanh)
        # softplus(-2x) = ln(1 + exp(-2x)); args are small so exp is safe
        spv = ppool.tile([128, 512], F32, tag="spv", bufs=1, name="spv")
        nc.scalar.activation(spv[:], hp_sb[:], AF.Exp, scale=-2.0)
        nc.scalar.activation(spv[:], spv[:], AF.Ln, bias=1.0)
        nlsc = ppool.tile([128, 512], F32, tag="nlsc", bufs=1, name="nlsc")
        nc.vector.tensor_scalar(out=nlsc[:], in0=hp_sb[:], scalar1=-1.0,
                                scalar2=3.0, op0=ALU.max, op1=ALU.min)
        # v = mu + softplus(-2 mu) on mu-rows (nls-rows unused)
        vmu = ppool.tile([128, 512], F32, tag="vmu", bufs=1, name="vmu")
        nc.vector.tensor_tensor(out=vmu[:], in0=hp_sb[:], in1=spv[:],
                                op=ALU.add)
        # logp = 2*sum(v over mu-rows) + sum(nlsc over nls-rows) + const
        psl = p2ps.tile([64, 512], F32, tag="p2", name="psl")
        _mm(nc, psl[0:NGRP, :], lsum_sb[:, 0:NGRP], vmu[:],
            start=True, stop=False)
        _mm(nc, psl[0:NGRP, :], lsum_sb[:, NGRP:2 * NGRP], nlsc[:],
            start=False, stop=True)
        lp_sb = ppool.tile([NGRP, 512], F32, tag="lp", bufs=1, name="lp_sb")
        nc.scalar.activation(lp_sb[:], psl[0:NGRP, :], AF.Identity,
                             bias=bias_sb[0:NGRP, 5:6])

        nc.sync.dma_start(out=th_o[:], in_=th[:])
        nc.sync.dma_start(out=lp_o[:], in_=lp_sb[:])
        nc.sync.dma_start(out=augt_o[:], in_=augt_sb[:])

    nc.compile()
    return nc


def _shared_inputs(m1_w, m2_w, fc1_w, fc2_w, mu_w, ls_w,
                   m1_b, m2_b, fc1_b, fc2_b, mu_b, ls_b):
    f = np.float32
    Wr, Ws = m1_w[:D].astype(f), m1_w[D:].astype(f)

    wrws = np.zeros((128, 128), f)
    wrws[0:32, 0:64] = Wr
    wrws[32:64, 0:64] = Ws
    wrws[64:96, 64:128] = Wr
    wrws[96:128, 64:128] = Ws

    m2bd = np.zeros((128, 128), f)
    m2bd[0:64, 0:64] = m2_w
    m2bd[64:128, 64:128] = m2_w

    i128 = np.eye(128, dtype=f)

    gmat = np.zeros((128, GRID), f)
    ii = np.repeat(np.arange(V), V)
    jj = np.tile(np.arange(V), V)
    col = np.arange(GRID)
    gmat[jj, col] = 1.0          # recv one-hot -> A part
    gmat[V + ii, col] = 1.0      # send one-hot -> B part

    hdw = np.concatenate([mu_w, -ls_w], axis=1).astype(f)   # [64, 16]

    lsum = np.zeros((128, 2 * NGRP), f)
    for g in range(NGRP):
        lsum[16 * g:16 * g + A, g] = 2.0               # 2*(mu + softplus)
        lsum[16 * g + A:16 * (g + 1), NGRP + g] = 1.0  # + (-log_std clipped)

    biasv = np.zeros((128, 8), f)
    biasv[:, 0] = np.concatenate([m1_b, m1_b])
    biasv[:, 1] = np.concatenate([m2_b, m2_b])
    biasv[0:64, 2] = fc1_b
    biasv[0:64, 3] = fc2_b
    biasv[0:2 * A, 4] = np.concatenate([mu_b, -ls_b])
    biasv[0:NGRP, 5] = -float(A) * (HALF_LOG_2PI + 2.0 * LOG2)

    return {
        "wrws": wrws, "m2bd": m2bd, "i128": i128, "gmat": gmat,
        "fc1w": fc1_w.astype(f), "fc2w": fc2_w.astype(f), "hdw": hdw,
        "lsum": lsum, "biasv": biasv,
    }


def _core_inputs(inputs_c, edges_c, send, recv):
    """Per-core marshalling: layout only (transpose / one-hot placement)."""
    f = np.float32
    XT = np.ascontiguousarray(inputs_c.transpose(0, 2, 1)).astype(f)  # [BC, D, V]

    xt4 = np.zeros((128, 128 * NPAIR), f)
    for p in range(NPAIR):
        c0 = 128 * p
        xt4[0:32, c0:c0 + 64] = XT[2 * p]
        xt4[32:64, c0 + 64:c0 + 128] = XT[2 * p]
        xt4[64:96, c0:c0 + 64] = XT[2 * p + 1]
        xt4[96:128, c0 + 64:c0 + 128] = XT[2 * p + 1]

    xtc = np.ascontiguousarray(inputs_c.reshape(TOK, D).T).astype(f)

    wvals = edges_c[:, :, 1].astype(f)                      # [BC, E]
    cols = (send.astype(np.int64) * V + recv.astype(np.int64))
    wg = np.zeros((BC, GRID), f)
    if len(np.unique(cols)) == cols.shape[0]:
        wg[:, cols] = wvals
    else:
        np.add.at(wg, (np.arange(BC)[:, None],
                       np.broadcast_to(cols, (BC, cols.shape[0]))), wvals)

    return {"xt4": xt4, "xtc": xtc, "wg": wg}


_NC_CACHE = {}


def _get_nc():
    if "nc" not in _NC_CACHE:
        _NC_CACHE["nc"] = build_nc()
    return _NC_CACHE["nc"]


def make_in_maps(inputs, edges, m1_w, m1_b, m2_w, m2_b, fc1_w, fc1_b,
                 fc2_w, fc2_b, mu_w, mu_b, ls_w, ls_b, send_e, recv_e):
    shared = _shared_inputs(m1_w, m2_w, fc1_w, fc2_w, mu_w, ls_w,
                            m1_b, m2_b, fc1_b, fc2_b, mu_b, ls_b)
    send = np.asarray(send_e)
    recv = np.asarray(recv_e)
    in_maps = []
    for c in range(NCORES):
        sl = slice(c * BC, (c + 1) * BC)
        m = dict(shared)
        m.update(_core_inputs(np.asarray(inputs)[sl], np.asarray(edges)[sl],
                              send, recv))
        in_maps.append(m)
    return in_maps


def unmarshal(results):
    """results: list of per-core dicts with augt/tanhpk/logppk."""
    tanh_out = np.empty((B, V, A), np.float32)
    logp_out = np.empty((B, V), np.float32)
    aug_out = np.empty((B, V, D + H), np.float32)
    for c, r in enumerate(results):
        sl = slice(c * BC, (c + 1) * BC)
        aug_out[sl] = r["augt"].T.reshape(BC, V, D + H)
        th = r["tanhpk"].reshape(NGRP, 16, 512)[:, :A, :]     # [8, 8, 512]
        tanh_out[sl] = th.transpose(0, 2, 1).reshape(BC, V, A)
        logp_out[sl] = r["logppk"].reshape(TOK).reshape(BC, V)
    return tanh_out, logp_out, aug_out


def kernel(inputs, edges, m1_w, m1_b, m2_w, m2_b, fc1_w, fc1_b,
           fc2_w, fc2_b, mu_w, mu_b, ls_w, ls_b, send_e, recv_e):
    nc = _get_nc()
    in_maps = make_in_maps(inputs, edges, m1_w, m1_b, m2_w, m2_b,
                           fc1_w, fc1_b, fc2_w, fc2_b, mu_w, mu_b,
                           ls_w, ls_b, send_e, recv_e)
    res = run_bass_kernel_spmd(nc, in_maps, list(range(NCORES)))
    return unmarshal(res.results)


# revision 13
# speedup vs baseline: 3.9092x; 3.9092x over previous
"""DNRI MLP decoder (GNN message passing) Trainium2 Bass kernel.

Sharding: data-parallel over batch (B=512 -> 64 per core x 8 cores).

Per-core device pipeline (all value arithmetic on device):
  - Node projections A = X @ m1_w[:D], B = X @ m1_w[D:] via one matmul per
    elem-pair using a host-prepared block-sparse X^T layout (xt4).
  - Edge gather Hpre'[b, (i,j)] = A[b, j] + B[b, i] over the dense V x V grid
    via matmul against a static one-hot matrix G.  Real edges map to grid
    cells (send, recv); edge weights land in w_grid (0 elsewhere), so absent
    cells and skipped (diagonal) cells contribute nothing downstream.
  - h = relu(Hpre' + b1)  (ScalarE drain)
  - msg = relu(h @ m2_w + b2)  (block-diag m2, ScalarE drain)
  - msgw = msg * w_grid-broadcast  (VectorE; edge weight >= 0 so exact)
  - agg[b, j] = sum_i msgw[b, (i,j)] : 8 PSUM-accumulating identity matmuls
    fold i-blocks mod 8, then a strided VectorE reduce finishes the sum.
  - aug = [X, agg]; fc1/fc2/mu/ls heads + tanh/softplus/logp epilogue.

Host does layout marshalling only (transposes, one-hot patterns, padding,
sharding); every floating-point op on tensor values runs on the NeuronCores.
(The only exception is summing duplicate-edge weights during w_grid
staging, which never triggers for the complete-graph edge list.)
"""

import os
import sys

import numpy as np

for _p in ("/opt/trn_rl_repo", "/root/.axon_site/_ro/trn_rl_repo"):
    if os.path.isdir(_p) and _p not in sys.path:
        sys.path.insert(0, _p)

import concourse.bass as bass
import concourse.mybir as mybir
from concourse import bacc, tile
from concourse.bass_utils import run_bass_kernel_spmd

B, V, D, H, A = 512, 64, 32, 64, 8
NCORES = 8
BC = B // NCORES            # 64 batch elems per core
NPAIR = BC // 2             # 32 elem pairs per core
GRID = V * V                # 4096 dense edge grid
TOK = BC * V                # 4096 tokens per core
NGRP = TOK // 512           # 8 token groups of 512
HALF_LOG_2PI = 0.5 * float(np.log(2.0 * np.pi))
LOG2 = float(np.log(2.0))

F32 = mybir.dt.float32
F32R = mybir.dt.float32r
AF = mybir.ActivationFunctionType
ALU = mybir.AluOpType

# matmul input dtype knob (float32r = full-rate fp32 path on TRN2 PE)
MM_DT = F32R


def _mm(npc, out, lhsT, rhs, **kw):
    npc.tensor.matmul(out, lhsT.bitcast(MM_DT), rhs.bitcast(MM_DT), **kw)


def build_nc():
    """Build the per-core Bass module (identical on all cores)."""
    nc = bacc.Bacc()

    # --- DRAM parameters -------------------------------------------------
    xt4_d = nc.declare_dram_parameter("xt4", [128, 128 * NPAIR], F32, isOutput=False)
    xtc_d = nc.declare_dram_parameter("xtc", [D, TOK], F32, isOutput=False)
    wg_d = nc.declare_dram_parameter("wg", [BC, GRID], F32, isOutput=False)
    g_d = nc.declare_dram_parameter("gmat", [128, GRID], F32, isOutput=False)
    wrws_d = nc.declare_dram_parameter("wrws", [128, 128], F32, isOutput=False)
    m2bd_d = nc.declare_dram_parameter("m2bd", [128, 128], F32, isOutput=False)
    i128_d = nc.declare_dram_parameter("i128", [128, 128], F32, isOutput=False)
    fc1w_d = nc.declare_dram_parameter("fc1w", [D + H, H], F32, isOutput=False)
    fc2w_d = nc.declare_dram_parameter("fc2w", [H, H], F32, isOutput=False)
    hdw_d = nc.declare_dram_parameter("hdw", [H, 2 * A], F32, isOutput=False)
    lsum_d = nc.declare_dram_parameter("lsum", [128, 2 * NGRP], F32, isOutput=False)
    bias_d = nc.declare_dram_parameter("biasv", [128, 8], F32, isOutput=False)

    augt_o = nc.declare_dram_parameter("augt", [D + H, TOK], F32, isOutput=True)
    th_o = nc.declare_dram_parameter("tanhpk", [128, 512], F32, isOutput=True)
    lp_o = nc.declare_dram_parameter("logppk", [NGRP, 512], F32, isOutput=True)

    with tile.TileContext(nc) as tc, \
         tc.tile_pool(name="const", bufs=1) as cpool, \
         tc.tile_pool(name="big", bufs=1) as bpool, \
         tc.tile_pool(name="work", bufs=2) as wpool, \
         tc.tile_pool(name="ph2", bufs=2) as ppool, \
         tc.tile_pool(name="hps", bufs=1, space="PSUM") as hps, \
         tc.tile_pool(name="m2ps", bufs=1, space="PSUM") as m2ps, \
         tc.tile_pool(name="scps", bufs=1, space="PSUM") as scps, \
         tc.tile_pool(name="p2ps", bufs=1, space="PSUM") as p2ps:

        # --- constants into SBUF ----------------------------------------
        g_sb = cpool.tile([128, GRID], F32, name="g_sb")
        nc.sync.dma_start(out=g_sb[:], in_=g_d[:])
        xt4_sb = cpool.tile([128, 128 * NPAIR], F32, name="xt4_sb")
        nc.sync.dma_start(out=xt4_sb[:], in_=xt4_d[:])
        wg_sb = cpool.tile([BC, GRID], F32, name="wg_sb")
        nc.sync.dma_start(out=wg_sb[:], in_=wg_d[:])
        wrws_sb = cpool.tile([128, 128], F32, name="wrws_sb")
        nc.sync.dma_start(out=wrws_sb[:], in_=wrws_d[:])
        m2bd_sb = cpool.tile([128, 128], F32, name="m2bd_sb")
        nc.sync.dma_start(out=m2bd_sb[:], in_=m2bd_d[:])
        i128_sb = cpool.tile([128, 128], F32, name="i128_sb")
        nc.sync.dma_start(out=i128_sb[:], in_=i128_d[:])
        fc1w_sb = cpool.tile([D + H, H], F32, name="fc1w_sb")
        nc.sync.dma_start(out=fc1w_sb[:], in_=fc1w_d[:])
        fc2w_sb = cpool.tile([H, H], F32, name="fc2w_sb")
        nc.sync.dma_start(out=fc2w_sb[:], in_=fc2w_d[:])
        hdw_sb = cpool.tile([H, 2 * A], F32, name="hdw_sb")
        nc.sync.dma_start(out=hdw_sb[:], in_=hdw_d[:])
        lsum_sb = cpool.tile([128, 2 * NGRP], F32, name="lsum_sb")
        nc.sync.dma_start(out=lsum_sb[:], in_=lsum_d[:])
        bias_sb = cpool.tile([128, 8], F32, name="bias_sb")
        nc.sync.dma_start(out=bias_sb[:], in_=bias_d[:])

        ab_sb = bpool.tile([128, 128 * NPAIR], F32, name="ab_sb")
        augt_sb = bpool.tile([D + H, TOK], F32, name="augt_sb")
        hp_sb = bpool.tile([128, 512], F32, name="hp_sb")
        # aug rows 0:32 = X features (token order matches column order)
        nc.sync.dma_start(out=augt_sb[0:D, :], in_=xtc_d[:])

        # --- node projections: AB_all[:, 128p:128p+128] per pair --------
        for gidx in range(NGRP):
            pps = m2ps.tile([128, 512], F32, tag="m2", name=f"pj{gidx}")
            for q in range(4):
                p = 4 * gidx + q
                _mm(nc, pps[:, 128 * q:128 * (q + 1)],
                    xt4_sb[:, 128 * p:128 * (p + 1)], wrws_sb[:],
                    start=True, stop=True)
            nc.scalar.activation(ab_sb[:, 512 * gidx:512 * (gidx + 1)], pps[:],
                                 AF.Copy)

        # --- main edge pipeline, one elem-pair at a time ----------------
        for p in range(NPAIR):
            ab_pair = ab_sb[:, 128 * p:128 * (p + 1)]

            # broadcast the two edge-weight rows across partition halves:
            # stage each row to partition 0/64 by DMA, then GPSIMD
            # partition_broadcast fans it out (start partition must be 0/64)
            w2_sb = wpool.tile([128, GRID], F32, tag="w2", bufs=1,
                               name=f"w2_{p}")
            nc.sync.dma_start(out=w2_sb[0:1, :], in_=wg_sb[2 * p:2 * p + 1, :])
            nc.sync.dma_start(out=w2_sb[64:65, :],
                              in_=wg_sb[2 * p + 1:2 * p + 2, :])
            nc.gpsimd.partition_broadcast(w2_sb[0:64, :], w2_sb[0:1, :],
                                          channels=64)
            nc.gpsimd.partition_broadcast(w2_sb[64:128, :], w2_sb[64:65, :],
                                          channels=64)

            h_sb = wpool.tile([128, GRID], F32, tag="h", name=f"h_{p}")
            for half in range(2):
                hpt = hps.tile([128, 2048], F32, tag="hps", name=f"hp{p}_{half}")
                for cc in range(4):
                    c = 4 * half + cc
                    _mm(nc, hpt[:, 512 * cc:512 * (cc + 1)], ab_pair,
                        g_sb[:, 512 * c:512 * (c + 1)], start=True, stop=True)
                nc.scalar.activation(h_sb[:, 2048 * half:2048 * (half + 1)],
                                     hpt[:], AF.Relu, bias=bias_sb[:, 0:1])

            msg_sb = wpool.tile([128, GRID], F32, tag="msg", name=f"msg_{p}")
            for dh in range(4):
                mpt = m2ps.tile([128, 1024], F32, tag="m2", name=f"mp{p}_{dh}")
                for cc in range(2):
                    c = 2 * dh + cc
                    _mm(nc, mpt[:, 512 * cc:512 * (cc + 1)], m2bd_sb[:],
                        h_sb[:, 512 * c:512 * (c + 1)], start=True, stop=True)
                nc.scalar.activation(msg_sb[:, 1024 * dh:1024 * (dh + 1)],
                                     mpt[:], AF.Relu, bias=bias_sb[:, 1:2])

            # msgw = msg * w (in place)
            nc.vector.tensor_tensor(out=msg_sb[:], in0=msg_sb[:],
                                    in1=w2_sb[:], op=ALU.mult)

            # scatter-sum over i: 8 accumulating identity matmuls fold the
            # 64 i-blocks down to 8, then reduce the remaining 8.
            scp = scps.tile([128, 512], F32, tag="sc", name=f"sc_{p}")
            for c in range(8):
                _mm(nc, scp[:], i128_sb[:], msg_sb[:, 512 * c:512 * (c + 1)],
                    start=(c == 0), stop=(c == 7))
            aggsb = wpool.tile([128, V], F32, tag="agg", name=f"agg_{p}")
            nc.vector.tensor_reduce(
                out=aggsb[:],
                in_=scp[:].rearrange("p (a j) -> p j a", a=8),
                axis=mybir.AxisListType.X, op=ALU.add)

            # relocate agg halves into aug rows 32:96 (partition shift by DMA)
            nc.sync.dma_start(out=augt_sb[D:D + H, 128 * p:128 * p + 64],
                              in_=aggsb[0:64, :])
            nc.sync.dma_start(out=augt_sb[D:D + H, 128 * p + 64:128 * p + 128],
                              in_=aggsb[64:128, :])

            # --- phase 2 on each finished 512-token group ---------------
            if p % 4 == 3:
                g = p // 4
                ps1 = p2ps.tile([64, 512], F32, tag="p2", name=f"f1{g}")
                _mm(nc, ps1[0:64, :], fc1w_sb[:],
                    augt_sb[:, 512 * g:512 * (g + 1)], start=True, stop=True)
                h1 = ppool.tile([64, 512], F32, tag="h1", name=f"h1_{g}")
                nc.scalar.activation(h1[:], ps1[0:64, :], AF.Relu,
                                     bias=bias_sb[0:64, 2:3])
                ps2 = p2ps.tile([64, 512], F32, tag="p2", name=f"f2{g}")
                _mm(nc, ps2[0:64, :], fc2w_sb[:], h1[:], start=True, stop=True)
                h2 = ppool.tile([64, 512], F32, tag="h2", name=f"h2_{g}")
                nc.scalar.activation(h2[:], ps2[0:64, :], AF.Relu,
                                     bias=bias_sb[0:64, 3:4])
                ps3 = p2ps.tile([64, 512], F32, tag="p2", name=f"hd{g}")
                _mm(nc, ps3[0:2 * A, :], hdw_sb[:], h2[:], start=True, stop=True)
                hd = ppool.tile([2 * A, 512], F32, tag="hd", name=f"hd_{g}")
                nc.scalar.activation(hd[:], ps3[0:2 * A, :], AF.Identity,
                                     bias=bias_sb[0:2 * A, 4:5])
                # repack into [128, 512]: group g -> partitions 16g:16g+16
                nc.sync.dma_start(out=hp_sb[16 * g:16 * (g + 1), :], in_=hd[:])

        # --- epilogue: tanh / softplus / clip / logp --------------------
        th = ppool.tile([128, 512], F32, tag="th", bufs=1, name="th")
        nc.scalar.activation(th[:], hp_sb[:], AF.Tanh)
        # softplus(-2x) = ln(1 + exp(-2x)); args are small so exp is safe
        spv = ppool.tile([128, 512], F32, tag="spv", bufs=1, name="spv")
        nc.scalar.activation(spv[:], hp_sb[:], AF.Exp, scale=-2.0)
        nc.scalar.activation(spv[:], spv[:], AF.Ln, bias=1.0)
        nlsc = ppool.tile([128, 512], F32, tag="nlsc", bufs=1, name="nlsc")
        nc.vector.tensor_scalar(out=nlsc[:], in0=hp_sb[:], scalar1=-1.0,
                                scalar2=3.0, op0=ALU.max, op1=ALU.min)
        # v = mu + softplus(-2 mu) on mu-rows (nls-rows unused)
        vmu = ppool.tile([128, 512], F32, tag="vmu", bufs=1, name="vmu")
        nc.vector.tensor_tensor(out=vmu[:], in0=hp_sb[:], in1=spv[:],
                                op=ALU.add)
        # logp = 2*sum(v over mu-rows) + sum(nlsc over nls-rows) + const
        psl = p2ps.tile([64, 512], F32, tag="p2", name="psl")
        _mm(nc, psl[0:NGRP, :], lsum_sb[:, 0:NGRP], vmu[:],
            start=True, stop=False)
        _mm(nc, psl[0:NGRP, :], lsum_sb[:, NGRP:2 * NGRP], nlsc[:],
            start=False, stop=True)
        lp_sb = ppool.tile([NGRP, 512], F32, tag="lp", bufs=1, name="lp_sb")
        nc.scalar.activation(lp_sb[:], psl[0:NGRP, :], AF.Identity,
                             bias=bias_sb[0:NGRP, 5:6])

        nc.sync.dma_start(out=th_o[:], in_=th[:])
        nc.sync.dma_start(out=lp_o[:], in_=lp_sb[:])
        nc.sync.dma_start(out=augt_o[:], in_=augt_sb[:])

    nc.compile()
    return nc


def _shared_inputs(m1_w, m2_w, fc1_w, fc2_w, mu_w, ls_w,
                   m1_b, m2_b, fc1_b, fc2_b, mu_b, ls_b):
    f = np.float32
    Wr, Ws = m1_w[:D].astype(f), m1_w[D:].astype(f)

    wrws = np.zeros((128, 128), f)
    wrws[0:32, 0:64] = Wr
    wrws[32:64, 0:64] = Ws
    wrws[64:96, 64:128] = Wr
    wrws[96:128, 64:128] = Ws

    m2bd = np.zeros((128, 128), f)
    m2bd[0:64, 0:64] = m2_w
    m2bd[64:128, 64:128] = m2_w

    i128 = np.eye(128, dtype=f)

    gmat = np.zeros((128, GRID), f)
    ii = np.repeat(np.arange(V), V)
    jj = np.tile(np.arange(V), V)
    col = np.arange(GRID)
    gmat[jj, col] = 1.0          # recv one-hot -> A part
    gmat[V + ii, col] = 1.0      # send one-hot -> B part

    hdw = np.concatenate([mu_w, -ls_w], axis=1).astype(f)   # [64, 16]

    lsum = np.zeros((128, 2 * NGRP), f)
    for g in range(NGRP):
        lsum[16 * g:16 * g + A, g] = 2.0               # 2*(mu + softplus)
        lsum[16 * g + A:16 * (g + 1), NGRP + g] = 1.0  # + (-log_std clipped)

    biasv = np.zeros((128, 8), f)
    biasv[:, 0] = np.concatenate([m1_b, m1_b])
    biasv[:, 1] = np.concatenate([m2_b, m2_b])
    biasv[0:64, 2] = fc1_b
    biasv[0:64, 3] = fc2_b
    biasv[0:2 * A, 4] = np.concatenate([mu_b, -ls_b])
    biasv[0:NGRP, 5] = -float(A) * (HALF_LOG_2PI + 2.0 * LOG2)

    return {
        "wrws": wrws, "m2bd": m2bd, "i128": i128, "gmat": gmat,
        "fc1w": fc1_w.astype(f), "fc2w": fc2_w.astype(f), "hdw": hdw,
        "lsum": lsum, "biasv": biasv,
    }


def _core_inputs(inputs_c, edges_c, send, recv):
    """Per-core marshalling: layout only (transpose / one-hot placement)."""
    f = np.float32
    XT = np.ascontiguousarray(inputs_c.transpose(0, 2, 1)).astype(f)  # [BC, D, V]

    xt4 = np.zeros((128, 128 * NPAIR), f)
    for p in range(NPAIR):
        c0 = 128 * p
        xt4[0:32, c0:c0 + 64] = XT[2 * p]
        xt4[32:64, c0 + 64:c0 + 128] = XT[2 * p]
        xt4[64:96, c0:c0 + 64] = XT[2 * p + 1]
        xt4[96:128, c0 + 64:c0 + 128] = XT[2 * p + 1]

    xtc = np.ascontiguousarray(inputs_c.reshape(TOK, D).T).astype(f)

    wvals = edges_c[:, :, 1].astype(f)                      # [BC, E]
    cols = (send.astype(np.int64) * V + recv.astype(np.int64))
    wg = np.zeros((BC, GRID), f)
    if len(np.unique(cols)) == cols.shape[0]:
        wg[:, cols] = wvals
    else:
        np.add.at(wg, (np.arange(BC)[:, None],
                       np.broadcast_to(cols, (BC, cols.shape[0]))), wvals)

    return {"xt4": xt4, "xtc": xtc, "wg": wg}


_NC_CACHE = {}


def _get_nc():
    if "nc" not in _NC_CACHE:
        _NC_CACHE["nc"] = build_nc()
    return _NC_CACHE["nc"]


def make_in_maps(inputs, edges, m1_w, m1_b, m2_w, m2_b, fc1_w, fc1_b,
                 fc2_w, fc2_b, mu_w, mu_b, ls_w, ls_b, send_e, recv_e):
    shared = _shared_inputs(m1_w, m2_w, fc1_w, fc2_w, mu_w, ls_w,
                            m1_b, m2_b, fc1_b, fc2_b, mu_b, ls_b)
    send = np.asarray(send_e)
    recv = np.asarray(recv_e)
    in_maps = []
    for c in range(NCORES):
        sl = slice(c * BC, (c + 1) * BC)
        m = dict(shared)
        m.update(_core_inputs(np.asarray(inputs)[sl], np.asarray(edges)[sl],
                              send, recv))
        in_maps.append(m)
    return in_maps


def unmarshal(results):
    """results: list of per-core dicts with augt/tanhpk/logppk."""
    tanh_out = np.empty((B, V, A), np.float32)
    logp_out = np.empty((B, V), np.float32)
    aug_out = np.empty((B, V, D + H), np.float32)
    for c, r in enumerate(results):
        sl = slice(c * BC, (c + 1) * BC)
        aug_out[sl] = r["augt"].T.reshape(BC, V, D + H)
        th = r["tanhpk"].reshape(NGRP, 16, 512)[:, :A, :]     # [8, 8, 512]
        tanh_out[sl] = th.transpose(0, 2, 1).reshape(BC, V, A)
        logp_out[sl] = r["logppk"].reshape(TOK).reshape(BC, V)
    return tanh_out, logp_out, aug_out


def kernel(inputs, edges, m1_w, m1_b, m2_w, m2_b, fc1_w, fc1_b,
           fc2_w, fc2_b, mu_w, mu_b, ls_w, ls_b, send_e, recv_e):
    nc = _get_nc()
    in_maps = make_in_maps(inputs, edges, m1_w, m1_b, m2_w, m2_b,
                           fc1_w, fc1_b, fc2_w, fc2_b, mu_w, mu_b,
                           ls_w, ls_b, send_e, recv_e)
    res = run_bass_kernel_spmd(nc, in_maps, list(range(NCORES)))
    return unmarshal(res.results)
